# revision 12
# baseline (speedup 1.0000x reference)
"""Trainium2 Bass kernel for nn_AtomicNeuralNetwork (species-routed per-atom MLP).

Math (per frame n, atom a with species s = numbers[a]):
    h1 = silu(W1[s].T x + b1[s]);  h2 = silu(W2[s].T h1 + b2[s]);  out = W3[s].T h2 + b3[s]
Shapes: N=4096 frames, A=256 atoms, D_IN=39, H=50, S=8 species.

Strategy (v3 "block-diagonal, bias-in-matmul, paired ACTs"):
  - Data parallel over frames: 512 frames per NeuronCore x 8 cores.
  - Host groups atoms into species-pure packs of 4 (2 column-halves x 2
    "pair" slots). An atom pair is stacked on SBUF partitions 0:39 / 39:78
    and multiplied by a BLOCK-DIAGONAL weight image, so one matmul
    computes both atoms' layer: 2 matmuls per pack per layer.
  - ScalarE silu is the bottleneck (~(FD+352)/1.2ns per instruction on
    HW). To amortize the 352-cycle constant, packs are processed in
    PAIRS: each ACT covers 2048 psum columns. Biases are folded into the
    matmul via a ones-row appended to the streamed desc (row 78) and a
    constant c0 with silu(c0)=1 that regenerates the ones-row in h1, so
    ACTs need no per-species bias AP. b3 is added on the host.
  - PSUM is managed manually as one [128,4096] tile = two 2048-col
    regions ping-ponged by pair parity. L2 overwrites L1's region after
    ACT1 consumed it (WAR handled by the tile framework); L3 (4
    accumulating matmuls, M=8 -> the pair's 8 atom slots) lands in the
    consumed region; DVE copies [8,512] to an SBUF out tile; one output
    DMA per group of 8 packs.
  - Emission is software-pipelined 3 deep (L1[q] | L2[q-1] | L3[q-2]) so
    ScalarE's strict FIFO queue never waits on a just-issued matmul.
  - Everything bf16 on the matmul path (PSUM accumulates fp32); desc is
    downcast to bf16 on the host (halves HBM traffic).
"""

import sys

for _p in ("/opt/trn_rl_repo",):
    if _p not in sys.path:
        sys.path.append(_p)

import numpy as np
import ml_dtypes

import concourse.bass as bass  # noqa: F401
import concourse.mybir as mybir
import concourse.tile as tile
from concourse import bacc
from concourse import bass_utils

N, A, D, H, S = 4096, 256, 39, 50, 8
NCORES = 8
NF = N // NCORES            # frames per core
GRP = 8                     # packs per DMA group (even; pairs never straddle)
MM_DT = mybir.dt.bfloat16
NP_MM = ml_dtypes.bfloat16
DR = 2 * D + 1              # streamed desc rows: atom pair + ones-row
M1 = 2 * H + 1              # L1 output partitions: pair H + ones slot
M2 = 2 * H                  # L2 output partitions

# x with silu(x) = 1, snapped to bf16; ones_val = analytic silu of it.
_C0 = float(np.asarray(1.2784645080566406, NP_MM))
_ONES_VAL = float(np.asarray(_C0 / (1.0 + np.exp(-_C0)), NP_MM))

LAST = {}


def _pack_atoms(species):
    """Group atom indices into species-pure packs of 4. Leftovers are
    filled with duplicate atoms (discarded on unshard). The pack list is
    padded to even length with a duplicate pack so pairs are uniform.
    Returns (slot_atoms [4*NPACK], pack_species [NPACK],
    slot_valid [4*NPACK])."""
    slot_atoms = []
    pack_species = []
    slot_valid = []
    for s in range(S):
        idxs = np.nonzero(species == s)[0].tolist()
        if not idxs:
            continue
        r = len(idxs) % 4
        if r:
            idxs.extend(idxs[-1:] * (4 - r))
        for i in range(0, len(idxs), 4):
            slot_atoms.extend(idxs[i:i + 4])
            pack_species.append(s)
            slot_valid.extend([True] * 4)
        if r:
            for k in range(4 - r):
                slot_valid[-1 - k] = False
    if len(pack_species) % 2:           # pad to even pack count
        slot_atoms.extend(slot_atoms[-4:])
        pack_species.append(pack_species[-1])
        slot_valid.extend([False] * 4)
    return np.array(slot_atoms), np.array(pack_species), np.array(slot_valid)


def _groups(npack):
    return [(g, min(GRP, npack - g * GRP)) for g in range((npack + GRP - 1) // GRP)]


def _build_program(pack_species, npack, repeat=0):
    import contextlib

    nc = bacc.Bacc("TRN2", target_bir_lowering=False, debug=False)

    groups = _groups(npack)
    ngrp = len(groups)
    npair = npack // 2

    desc_in = nc.dram_tensor("desc_in", [ngrp, DR, GRP * 2 * NF], MM_DT, kind="ExternalInput")
    w1_in = nc.dram_tensor("w1_in", [128, S * M1], MM_DT, kind="ExternalInput")
    w2_in = nc.dram_tensor("w2_in", [128, S * M2], MM_DT, kind="ExternalInput")
    w3_in = nc.dram_tensor("w3_in", [128, npair * 4 * 8], MM_DT, kind="ExternalInput")
    out = nc.dram_tensor("out", [ngrp, 8, (GRP // 2) * NF], mybir.dt.float32, kind="ExternalOutput")

    Silu = mybir.ActivationFunctionType.Silu
    F32 = mybir.dt.float32

    with tile.TileContext(nc) as tc:
        with (
            tc.tile_pool(name="const", bufs=1) as cpool,
            tc.tile_pool(name="dt", bufs=3) as dpool,
            tc.tile_pool(name="h1p", bufs=3) as h1pool,
            tc.tile_pool(name="h2p", bufs=3) as h2pool,
            tc.tile_pool(name="op", bufs=2) as opool,
            tc.tile_pool(name="ps", bufs=1, space="PSUM") as pspool,
        ):
            w1 = cpool.tile([128, S * M1], MM_DT)
            w2 = cpool.tile([128, S * M2], MM_DT)
            w3 = cpool.tile([128, npair * 4 * 8], MM_DT)
            for t, src in ((w1, w1_in), (w2, w2_in), (w3, w3_in)):
                nc.sync.dma_start(t[:], src[:])

            loop_cm = tc.For_i(0, repeat, 1) if repeat else contextlib.nullcontext()
            with loop_cm:
                psB = pspool.tile([128, 4096], F32, tag="ps")
                X = psB[:, 0:2048]      # ps1 region (all pairs)
                Y = psB[:, 2048:4096]   # ps2 region; last 512 cols double as L3 out
                st = {}       # q -> dict(...)

                def stage1(q, g, jq, cur):
                    R = X
                    h1 = h1pool.tile([128, 2048], MM_DT, tag="h1")
                    for k in range(4):          # (pack-in-pair, colhalf)
                        pk, ch = divmod(k, 2)
                        p = g * GRP + 2 * jq + pk
                        s = int(pack_species[p])
                        c = (4 * jq + 2 * pk + ch) * NF
                        nc.tensor.matmul(
                            R[0:M1, k * NF:(k + 1) * NF],
                            w1[0:DR, s * M1:(s + 1) * M1],
                            cur[0:DR, c:c + NF],
                            start=True, stop=True)
                    nc.scalar.activation(h1[0:M1, :], R[0:M1, :], Silu)
                    st[q] = dict(g=g, jq=jq, h1=h1)

                def stage2(q):
                    d = st[q]
                    g, jq, h1 = d["g"], d["jq"], d["h1"]
                    R = Y
                    h2 = h2pool.tile([128, 2048], MM_DT, tag="h2")
                    for k in range(4):
                        pk, ch = divmod(k, 2)
                        p = g * GRP + 2 * jq + pk
                        s = int(pack_species[p])
                        nc.tensor.matmul(
                            R[0:M2, k * NF:(k + 1) * NF],
                            w2[0:M1, s * M2:(s + 1) * M2],
                            h1[0:M1, k * NF:(k + 1) * NF],
                            start=True, stop=True)
                    nc.scalar.activation(h2[0:M2, :], R[0:M2, :], Silu)
                    d["h2"] = h2

                gn_of = dict(groups)
                pocur = [None]

                def stage3(q):
                    d = st.pop(q)
                    g, jq, h2 = d["g"], d["jq"], d["h2"]
                    # L3 accumulates into Y's last 512-col block (bank 7),
                    # freed by ACT2[q]; L2[q+1]'s k3 matmul (emitted after
                    # the copy) overwrites it next.
                    R3 = Y[:, 3 * NF:4 * NF]
                    P = g * (GRP // 2) + jq     # global pair index
                    for k in range(4):
                        nc.tensor.matmul(
                            R3[0:8, 0:NF],
                            w3[0:M2, (P * 4 + k) * 8:(P * 4 + k + 1) * 8],
                            h2[0:M2, k * NF:(k + 1) * NF],
                            start=(k == 0), stop=(k == 3),
                            skip_group_check=True)
                    if jq == 0:
                        pocur[0] = opool.tile([128, (GRP // 2) * NF], F32, tag="o",
                                              name="po")
                    po = pocur[0]
                    nc.vector.tensor_copy(po[0:8, jq * NF:(jq + 1) * NF], R3[0:8, 0:NF])
                    if jq == gn_of[g] // 2 - 1:
                        nc.sync.dma_start(out[g, :, :], po[0:8, :])

                u = 0
                for g, gn in groups:
                    gw = gn * 2 * NF
                    cur = dpool.tile([128, GRP * 2 * NF], MM_DT, tag="dt")
                    hr = DR // 2
                    nc.sync.dma_start(cur[0:hr, 0:gw], desc_in[g, 0:hr, 0:gw])
                    nc.gpsimd.dma_start(cur[hr:DR, 0:gw], desc_in[g, hr:DR, 0:gw])
                    for jq in range(gn // 2):
                        # PE queue per iter: L1(u) | L3(u-2)+copy | L2(u-1).
                        # L3 must precede L2 (both touch Y's bank 7).
                        stage1(u, g, jq, cur)
                        if u >= 2:
                            stage3(u - 2)
                        if u >= 1:
                            stage2(u - 1)
                        u += 1
                stage3(u - 2)
                stage2(u - 1)
                stage3(u - 1)

    nc.compile()
    return nc


def _host_inputs(desc, numbers, W1, b1, W2, b2, W3, b3):
    desc = np.asarray(desc, dtype=np.float32)
    numbers = np.asarray(numbers).astype(np.int64)
    W1 = np.asarray(W1, np.float32); b1 = np.asarray(b1, np.float32)
    W2 = np.asarray(W2, np.float32); b2 = np.asarray(b2, np.float32)
    W3 = np.asarray(W3, np.float32); b3 = np.asarray(b3, np.float32)

    slot_atoms, pack_species, slot_valid = _pack_atoms(numbers)
    npack = len(pack_species)
    nslot = 4 * npack
    groups = _groups(npack)
    ngrp = len(groups)
    npair = npack // 2

    w1img = np.zeros((128, S * M1), np.float32)
    w2img = np.zeros((128, S * M2), np.float32)
    for s in range(S):
        c0 = s * M1
        w1img[0:D, c0:c0 + H] = W1[s]
        w1img[D:2 * D, c0 + H:c0 + M2] = W1[s]
        w1img[2 * D, c0:c0 + H] = b1[s]
        w1img[2 * D, c0 + H:c0 + M2] = b1[s]
        w1img[2 * D, c0 + M2] = _C0
        c0 = s * M2
        w2img[0:H, c0:c0 + H] = W2[s]
        w2img[H:M2, c0 + H:c0 + M2] = W2[s]
        w2img[M2, c0:c0 + H] = b2[s] / _ONES_VAL
        w2img[M2, c0 + H:c0 + M2] = b2[s] / _ONES_VAL

    w3img = np.zeros((128, npair * 4 * 8), np.float32)
    for P in range(npair):
        for k in range(4):
            pk, ch = divmod(k, 2)
            s = int(pack_species[2 * P + pk])
            base = (P * 4 + k) * 8
            col = 4 * pk + 2 * ch
            w3img[0:H, base + col] = W3[s, :, 0]
            w3img[H:M2, base + col + 1] = W3[s, :, 0]

    wmaps = {
        "w1_in": w1img.astype(NP_MM), "w2_in": w2img.astype(NP_MM),
        "w3_in": w3img.astype(NP_MM),
    }

    npack_pad = ngrp * GRP
    in_maps = []
    for c in range(NCORES):
        dc = desc[c * NF:(c + 1) * NF][:, slot_atoms, :]                  # [NF, NSLOT, D]
        dc = np.ascontiguousarray(dc.transpose(1, 2, 0)).astype(NP_MM)    # [NSLOT, D, NF]
        if npack_pad != npack:
            pad = np.zeros((4 * (npack_pad - npack), D, NF), NP_MM)
            dc = np.concatenate([dc, pad], axis=0)
        dc = dc.reshape(ngrp, GRP, 2, 2, D, NF)      # g, j, ch, p, q, n
        dc = dc.transpose(0, 3, 4, 1, 2, 5)          # g, p, q, j, ch, n
        dc = dc.reshape(ngrp, 2 * D, GRP * 2 * NF)
        ones = np.ones((ngrp, 1, GRP * 2 * NF), NP_MM)
        dc = np.ascontiguousarray(np.concatenate([dc, ones], axis=1))
        in_maps.append({"desc_in": dc, **wmaps})
    return in_maps, slot_atoms, pack_species, slot_valid, npack, nslot


def kernel(desc, numbers, W1, b1, W2, b2, W3, b3):
    (in_maps, slot_atoms, pack_species, slot_valid,
     npack, nslot) = _host_inputs(desc, numbers, W1, b1, W2, b2, W3, b3)

    nc = _build_program(pack_species, npack)

    last_err = None
    for _attempt in range(3):
        try:
            res = bass_utils.run_bass_kernel_spmd(
                nc, in_maps, core_ids=list(range(NCORES)))
            break
        except Exception as e:  # transient axon terminal failures
            last_err = e
            import time
            time.sleep(20)
    else:
        raise last_err

    LAST.update(nc=nc, in_maps=in_maps, res=res, npack=npack)

    b3v = np.asarray(b3, np.float32)[np.asarray(numbers).astype(np.int64), 0]  # [A]
    ngrp = len(_groups(npack))
    out = np.empty((N, A), np.float32)
    for c in range(NCORES):
        oc = res.results[c]["out"]                     # [ngrp, 8, (GRP//2)*NF]
        # partition k of pair-slot jq = pack 2*jq + k//4, slot k%4
        oc = oc.reshape(ngrp, 8, GRP // 2, NF)         # g, k, jq, n
        oc = oc.transpose(0, 2, 1, 3).reshape(-1, NF)  # (g, jq, k) slot-major
        out[c * NF:(c + 1) * NF, slot_atoms[slot_valid]] = oc[:nslot][slot_valid].T
    out += b3v[None, :]
    return out
